# revision 2
# baseline (speedup 1.0000x reference)
"""Trainium2 Bass kernel for a dense transformer layer (attention + FFN, LN over seq dim).

Sharding: sequence-parallel over 8 NeuronCores. Each core handles SL=256 of the
S=2048 sequence positions (for all batches). K and V are all-gathered (bf16);
LayerNorm over the sequence dim uses tiny all-reduced sum/sumsq stats.
All compute is done in a transposed orientation ([dim, batch*seq]) so every
matmul contracts over the partition axis with no large on-chip transposes.
"""
import os
import sys

sys.path.insert(0, "/opt/trn_rl_repo")

from contextlib import ExitStack

import numpy as np
import ml_dtypes

import concourse.bass as bass
import concourse.tile as tile
from concourse import bacc, mybir
from concourse.bass import ds, ts
from concourse.bass_utils import run_bass_kernel_spmd
from concourse.masks import make_identity
from concourse.kernels.tile_matmul import (
    ShapeInfo,
    composable_matmul_tile_kernel,
    dma_from_dram_kxm,
    dma_from_dram_kxn,
    dma_to_dram_mxn,
    k_pool_min_bufs,
    scalar_copyback,
)

# Problem constants (hardcoded per spec)
R = 8          # cores
B = 4          # batch
S = 2048       # sequence
SL = S // R    # local sequence rows per core = 256
E = 2048       # embed
H = 16         # heads
D = 128        # head dim
HD = H * D     # = E
F = 4 * E      # ffn hidden = 8192
NL = B * SL    # local "n" free dim = 1024
P = 128
ET = E // P    # 16
FT = F // P    # 64
TT = S // P    # 16 key tiles
EPS = 1e-5
ISQD = 1.0 / float(np.sqrt(D))

BF = mybir.dt.bfloat16
F32 = mybir.dt.float32
AX = mybir.AxisListType
ALU = mybir.AluOpType
ACT = mybir.ActivationFunctionType

_STATE = {}

LAST_EXEC_NS = None


def _install_ntff_hook():
    """Provide antenv.axon_hooks (missing in this image) so trace=True works.

    Replicates trn_agent_boot.trn_boot._ntff_profile_via_ctypes against the
    injected libaxon_pjrt.so. No-op if already present or the .so is missing.
    """
    import contextlib
    import ctypes
    import types

    try:
        from antenv.axon_hooks import get_axon_ntff_profile_hook  # noqa: F401

        return
    except ImportError:
        pass
    so_path = "/opt/axon/libaxon_pjrt.so"
    hook = None
    if os.path.exists(so_path):
        lib = ctypes.CDLL(so_path)
        if hasattr(lib, "axon_start_nrt_profile"):
            lib.axon_start_nrt_profile.argtypes = [
                ctypes.POINTER(ctypes.c_int64),
                ctypes.c_size_t,
            ]
            lib.axon_start_nrt_profile.restype = ctypes.c_int64
            lib.axon_stop_nrt_profile.argtypes = [ctypes.c_char_p]
            lib.axon_stop_nrt_profile.restype = ctypes.c_int64

            @contextlib.contextmanager
            def _hook(output_dir, device_ids):
                import jax

                jax.devices()
                if device_ids:
                    ids = (ctypes.c_int64 * len(device_ids))(*device_ids)
                    rc = lib.axon_start_nrt_profile(ids, len(device_ids))
                else:
                    rc = lib.axon_start_nrt_profile(None, 0)
                if rc != 0:
                    raise RuntimeError(f"axon_start_nrt_profile rc={rc}")
                try:
                    yield
                finally:
                    n = lib.axon_stop_nrt_profile(str(output_dir).encode())
                    print(f"profile: {n} ntff file(s) written to {output_dir}")

            hook = _hook

    import antenv

    mod = types.ModuleType("antenv.axon_hooks")
    mod.get_axon_ntff_profile_hook = lambda: hook
    mod.set_axon_ntff_profile_hook = lambda h: None
    antenv.axon_hooks = mod
    sys.modules["antenv.axon_hooks"] = mod

    # zero-egress container: don't try to upload trace artifacts
    import concourse.bass_utils as _bu

    _bu.upload_artifacts = lambda tmpdir: tmpdir


def _resident_kxn(sb):
    """kxn producer serving slices of an SBUF-resident [P, K//P, N] tile."""

    def prod(nc, md):
        return sb[
            :,
            ts(md.k_tile_idx, md.k_subtiles),
            ds(md.n_tile_idx * md.n_tile, md.n_tile),
        ]

    return prod


def _resident_kxm_batched(sb, m_offs):
    """kxm producer over an SBUF-resident tile with m-batches at given offsets."""

    def prod(nc, md):
        return sb[
            :,
            ts(md.k_tile_idx, md.k_subtiles),
            ds(m_offs[md.m_batch_idx] + md.m_tile_idx * md.m_tile, md.m_tile),
        ]

    return prod


def _bias_act_reducer(bias_sb, func):
    """PSUM->SBUF eviction fused with per-partition bias (+ activation func)."""

    def red(nc, psum, sbuf, md):
        m_abs = md.m_tile_idx * md.m_subtiles + md.m_subtile_idx
        nc.scalar.activation(sbuf, psum, func, bias=bias_sb[:, m_abs : m_abs + 1])

    return red


def _vbias_reducer(bv_sb):
    """PSUM->SBUF eviction fused with bias along the free (n) dim."""

    def red(nc, psum, sbuf, md):
        n0 = md.n_tile_idx * md.n_tile + md.n_subtile_idx * md.n_subtile
        w = psum.free_size()
        nc.vector.tensor_add(out=sbuf, in0=psum, in1=bv_sb[:, ds(n0, w)])

    return red


def build():
    nc = bacc.Bacc("TRN2", target_bir_lowering=False, debug=False, num_devices=R)

    # ---- external inputs (per-core views prepared on host) ----
    x_s = nc.dram_tensor("x_s", [B, SL, E], F32, kind="ExternalInput")
    WqT = nc.dram_tensor("WqT", [E, HD], BF, kind="ExternalInput")
    WkT = nc.dram_tensor("WkT", [E, HD], BF, kind="ExternalInput")
    WvT = nc.dram_tensor("WvT", [E, HD], BF, kind="ExternalInput")
    WoT = nc.dram_tensor("WoT", [E, E], BF, kind="ExternalInput")
    W1T = nc.dram_tensor("W1T", [E, F], BF, kind="ExternalInput")
    W2T = nc.dram_tensor("W2T", [F, E], BF, kind="ExternalInput")
    bq_c = nc.dram_tensor("bq_c", [P, H], F32, kind="ExternalInput")
    bk_c = nc.dram_tensor("bk_c", [P, H], F32, kind="ExternalInput")
    bv_r = nc.dram_tensor("bv_r", [1, HD], F32, kind="ExternalInput")
    bo_c = nc.dram_tensor("bo_c", [P, ET], F32, kind="ExternalInput")
    b1_c = nc.dram_tensor("b1_c", [P, FT], F32, kind="ExternalInput")
    b2_c = nc.dram_tensor("b2_c", [P, ET], F32, kind="ExternalInput")

    # ---- internals ----
    kT_loc = nc.dram_tensor("kT_loc", [HD, NL], BF)
    kT_full = nc.dram_tensor("kT_full", [R * HD, NL], BF, addr_space="Shared")
    v_loc = nc.dram_tensor("v_loc", [SL, B, HD], BF)
    v_full = nc.dram_tensor("v_full", [R * SL, B, HD], BF, addr_space="Shared")
    x1T = nc.dram_tensor("x1T", [E, NL], BF)
    hT = nc.dram_tensor("hT", [F, NL], BF)
    y2T = nc.dram_tensor("y2T", [E, NL], F32)
    st1_loc = nc.dram_tensor("st1_loc", [P, 2, ET, B], F32)
    st1_full = nc.dram_tensor("st1_full", [P, 2, ET, B], F32, addr_space="Shared")
    st2_loc = nc.dram_tensor("st2_loc", [P, 2, ET, B], F32)
    st2_full = nc.dram_tensor("st2_full", [P, 2, ET, B], F32, addr_space="Shared")
    outT = nc.dram_tensor("outT", [E, NL], F32, kind="ExternalOutput")

    rg = [list(range(R))]

    with tile.TileContext(nc, pool_alloc_mode="queue") as tc, ExitStack() as CTX:
        consts = CTX.enter_context(tc.tile_pool(name="consts", bufs=1))
        ident = consts.tile([P, P], BF)
        make_identity(nc, ident)
        ones_bf = consts.tile([P, 1], BF)
        nc.vector.memset(ones_bf, 1.0)
        eps_sb = consts.tile([P, 1], F32)
        nc.vector.memset(eps_sb, EPS)
        bq_sb = consts.tile([P, H], F32)
        nc.sync.dma_start(out=bq_sb[:], in_=bq_c[:])
        bk_sb = consts.tile([P, H], F32)
        nc.sync.dma_start(out=bk_sb[:], in_=bk_c[:])
        bo_sb = consts.tile([P, ET], F32)
        nc.sync.dma_start(out=bo_sb[:], in_=bo_c[:])
        b1_sb = consts.tile([P, FT], F32)
        nc.sync.dma_start(out=b1_sb[:], in_=b1_c[:])
        b2_sb = consts.tile([P, ET], F32)
        nc.sync.dma_start(out=b2_sb[:], in_=b2_c[:])
        bv_sb = consts.tile([P, HD], F32)
        nc.sync.dma_start(out=bv_sb[:], in_=bv_r[0:1, :].to_broadcast([P, HD]))

        # ---------- Phase A: x_s -> SBUF-resident x^T (bf16) ----------
        xsT_ctx = ExitStack()
        xsT_pool = xsT_ctx.enter_context(tc.tile_pool(name="xsT", bufs=1))
        xsT = xsT_pool.tile([P, ET, NL], BF)
        # et-outer order: each xsT e-tile completes early so the k-projection's
        # first contraction tiles can start before the whole transpose finishes.
        with tc.tile_pool(name="phA", bufs=1) as pa, tc.tile_pool(
            name="phA_xa", bufs=3
        ) as pa_xa, tc.tile_pool(name="phA_ps", bufs=4, space="PSUM") as pa_ps:
            xbs = []
            for b in range(B):
                for st in range(SL // P):
                    xa = pa_xa.tile([P, E], F32, tag="xa")
                    nc.sync.dma_start(out=xa[:], in_=x_s[b, st * P : (st + 1) * P, :])
                    xb = pa.tile([P, E], BF, tag=f"xb_{b}_{st}")
                    nc.vector.tensor_copy(out=xb[:], in_=xa[:])
                    xbs.append((b, st, xb))
            for et in range(ET):
                for b, st, xb in xbs:
                    pt = pa_ps.tile([P, P], BF, tag="pt")
                    nc.tensor.transpose(pt[:], xb[:, et * P : (et + 1) * P], ident)
                    nc.vector.tensor_copy(
                        out=xsT[:, et, ds(b * SL + st * P, P)], in_=pt[:]
                    )

        xsT_shape = ShapeInfo(pdims=((P, ET),), fdims=(NL,))

        # q projection output and attention output stay SBUF-resident
        qo_ctx = ExitStack()
        qo_pool = qo_ctx.enter_context(tc.tile_pool(name="qo_sb", bufs=1))
        qT_sb = qo_pool.tile([P, ET, NL], BF)
        oT_sb = qo_pool.tile([P, ET, NL], BF)

        def q_reducer(nc_, psum, sbuf, md):
            m_abs = md.m_tile_idx * md.m_subtiles + md.m_subtile_idx
            n0 = md.n_tile_idx * md.n_tile + md.n_subtile_idx * md.n_subtile
            nc_.vector.tensor_copy(
                out=qT_sb[:, m_abs, ds(n0, psum.free_size())], in_=psum
            )

        # ---------- Phase B: projections (k -> AG(k) -> v -> AG(v) -> q) ----------
        with ExitStack() as ctxB:
            wqk_pool = ctxB.enter_context(
                tc.tile_pool(name="w_qk", bufs=k_pool_min_bufs(WkT[:]))
            )
            kxm_prod_k, kxm_shape_k = dma_from_dram_kxm(wqk_pool, WkT[:])
            composable_matmul_tile_kernel(
                tc=tc,
                psum_n_bufs=2,
                kxm_shape=kxm_shape_k,
                kxn_shape=xsT_shape,
                output_type=BF,
                kxm_producer=kxm_prod_k,
                kxn_producer=_resident_kxn(xsT),
                mxn_consumer=dma_to_dram_mxn(kT_loc[:]),
                mxn_subtile_reducer=scalar_copyback(),
            )
            nc.gpsimd.collective_compute(
                "AllGather",
                ALU.bypass,
                replica_groups=rg,
                ins=[kT_loc[:]],
                outs=[kT_full[:]],
            )
            # v projection (natural [s, d] layout), one call batched over b
            vpool = ctxB.enter_context(
                tc.tile_pool(name="w_v", bufs=k_pool_min_bufs(WvT[:]))
            )
            kxn_prod_v, kxn_shape_v = dma_from_dram_kxn(vpool, WvT[:])
            v_consumers = [dma_to_dram_mxn(v_loc[:, b, :]) for b in range(B)]

            def v_consumer(nc_, mxn_tile, md):
                from dataclasses import replace

                v_consumers[md.m_batch_idx](nc_, mxn_tile, replace(md, m_batch_idx=0))

            composable_matmul_tile_kernel(
                tc=tc,
                psum_n_bufs=2,
                kxm_shape=ShapeInfo(pdims=((P, ET),), fdims=(SL,) * B),
                kxn_shape=kxn_shape_v,
                output_type=BF,
                kxm_producer=_resident_kxm_batched(xsT, [b * SL for b in range(B)]),
                kxn_producer=kxn_prod_v,
                mxn_consumer=v_consumer,
                mxn_subtile_reducer=_vbias_reducer(bv_sb),
            )
            nc.gpsimd.collective_compute(
                "AllGather",
                ALU.bypass,
                replica_groups=rg,
                ins=[v_loc[:]],
                outs=[v_full[:]],
            )
            kxm_prod_q, kxm_shape_q = dma_from_dram_kxm(wqk_pool, WqT[:])
            composable_matmul_tile_kernel(
                tc=tc,
                psum_n_bufs=2,
                kxm_shape=kxm_shape_q,
                kxn_shape=xsT_shape,
                output_type=BF,
                kxm_producer=kxm_prod_q,
                kxn_producer=_resident_kxn(xsT),
                mxn_consumer=lambda nc_, mxn_tile, md: None,
                mxn_subtile_reducer=q_reducer,
            )

        # ---------- Phase C: attention (per head) ----------
        with ExitStack() as ctxA:
            ap_kth = ctxA.enter_context(tc.tile_pool(name="att_kth", bufs=2))
            ap_v = ctxA.enter_context(tc.tile_pool(name="att_v", bufs=2))
            ap_sb = ctxA.enter_context(tc.tile_pool(name="att_sb", bufs=3))
            ap_pT = ctxA.enter_context(tc.tile_pool(name="att_pT", bufs=2))
            ps_l = ctxA.enter_context(tc.tile_pool(name="att_psl", bufs=3, space="PSUM"))
            ps_o = ctxA.enter_context(tc.tile_pool(name="att_pso", bufs=2, space="PSUM"))
            ps_d = ctxA.enter_context(tc.tile_pool(name="att_psd", bufs=2, space="PSUM"))
            for h in range(H):
                kth = ap_kth.tile([P, R, NL], BF, tag="kth")
                for r in range(R):
                    nc.sync.dma_start(
                        out=kth[:, r, :],
                        in_=kT_full[r * HD + h * P : r * HD + (h + 1) * P, :],
                    )
                nc.vector.tensor_scalar_add(kth[:], kth[:], bk_sb[:, h : h + 1])
                qh = ap_sb.tile([P, NL], BF, tag="qh")
                nc.vector.tensor_scalar_add(qh[:], qT_sb[:, h, :], bq_sb[:, h : h + 1])
                vball = ap_v.tile([P, TT, B, P], BF, tag="vball")
                for tt in range(TT):
                    nc.sync.dma_start(
                        out=vball[:, tt, :, :],
                        in_=v_full[tt * P : (tt + 1) * P, :, h * P : (h + 1) * P],
                    )
                for b in range(B):
                    pT = ap_pT.tile([P, TT, SL], BF, tag="pT")
                    for tt in range(TT):
                        r_i, sl0 = divmod(tt * P, SL)
                        pl = ps_l.tile([P, SL], F32, tag="pl")
                        nc.tensor.matmul(
                            pl[:],
                            lhsT=kth[:, r_i, ds(b * SL + sl0, P)],
                            rhs=qh[:, ds(b * SL, SL)],
                            start=True,
                            stop=True,
                        )
                        nc.scalar.activation(
                            pT[:, tt, :], pl[:], ACT.Exp, scale=ISQD
                        )
                    dd = ps_d.tile([1, SL], F32, tag="dd")
                    for tt in range(TT):
                        nc.tensor.matmul(
                            dd[:], lhsT=ones_bf[:], rhs=pT[:, tt, :],
                            start=(tt == 0), stop=(tt == TT - 1),
                        )
                    od = ps_o.tile([P, SL], F32, tag="od")
                    for tt in range(TT):
                        nc.tensor.matmul(
                            od[:], lhsT=vball[:, tt, b, :], rhs=pT[:, tt, :],
                            start=(tt == 0), stop=(tt == TT - 1),
                        )
                    rec = ap_sb.tile([1, SL], F32, tag="rec")
                    nc.vector.reciprocal(rec[:], dd[:])
                    recb = ap_sb.tile([P, SL], F32, tag="recb")
                    nc.gpsimd.partition_broadcast(recb[:], rec[:])
                    nc.vector.tensor_mul(oT_sb[:, h, ds(b * SL, SL)], od[:], recb[:])

        # ---------- Phase D: Wo + residual + inline LN1 stats -> y1T (SBUF) ----------
        # y1 accumulator + LN1 stats; closed after LN1, before xsT (LIFO order)
        y1_ctx = ExitStack()
        y1_pool = y1_ctx.enter_context(tc.tile_pool(name="y1sb", bufs=1))
        y1sb = y1_pool.tile([P, ET, NL], BF)
        st1_sb = y1_pool.tile([P, 2, ET, B], F32)

        with ExitStack() as ctxD:
            wo_pool = ctxD.enter_context(
                tc.tile_pool(name="w_wo", bufs=k_pool_min_bufs(WoT[:]))
            )
            cons_pool = ctxD.enter_context(tc.tile_pool(name="wo_cons", bufs=3))
            kxm_prod, kxm_shape = dma_from_dram_kxm(wo_pool, WoT[:])
            kxn_prod, kxn_shape = _resident_kxn(oT_sb), xsT_shape

            def wo_consumer(nc_, mxn_tile, md):
                c = md.n_tile_idx
                w = md.n_slice_size
                for sub in range(md.m_subtiles):
                    m_abs = md.m_tile_idx * md.m_subtiles + sub
                    dst = y1sb[:, m_abs, ds(c * 512, w)]
                    nc_.vector.tensor_add(
                        out=dst,
                        in0=mxn_tile[:, sub, :w],
                        in1=xsT[:, m_abs, ds(c * 512, w)],
                    )
                    nc_.vector.tensor_reduce(
                        out=st1_sb[:, 0, m_abs, 2 * c : 2 * c + 2],
                        in_=dst.rearrange("p (b s) -> p b s", b=2),
                        axis=AX.X,
                        op=ALU.add,
                    )
                    sq = cons_pool.tile([P, 512], F32, tag="wo_sq")
                    nc_.scalar.activation(sq[:, :w], dst, ACT.Square)
                    nc_.vector.tensor_reduce(
                        out=st1_sb[:, 1, m_abs, 2 * c : 2 * c + 2],
                        in_=sq[:, :w].rearrange("p (b s) -> p b s", b=2),
                        axis=AX.X,
                        op=ALU.add,
                    )

            composable_matmul_tile_kernel(
                tc=tc,
                psum_n_bufs=2,
                kxm_shape=kxm_shape,
                kxn_shape=kxn_shape,
                output_type=F32,
                kxm_producer=kxm_prod,
                kxn_producer=kxn_prod,
                mxn_consumer=wo_consumer,
                mxn_subtile_reducer=_bias_act_reducer(bo_sb, ACT.Identity),
            )

        # ---------- Phase E: LN1 (stats AR + normalize) -> x1T (DRAM, bf16) ----------
        with ExitStack() as ctxE:
            lnp = ctxE.enter_context(tc.tile_pool(name="ln1", bufs=4))
            nc.sync.dma_start(out=st1_loc[:], in_=st1_sb[:])
            nc.gpsimd.collective_compute(
                "AllReduce", ALU.add, replica_groups=rg,
                ins=[st1_loc[:]], outs=[st1_full[:]],
            )
            stf = lnp.tile([P, 2, ET, B], F32, tag="stf")
            nc.sync.dma_start(out=stf[:], in_=st1_full[:])
            r1 = lnp.tile([P, ET, B], F32, tag="r1")
            n1 = lnp.tile([P, ET, B], F32, tag="n1")
            mu = lnp.tile([P, ET, B], F32, tag="mu")
            var = lnp.tile([P, ET, B], F32, tag="var")
            nc.vector.tensor_scalar_mul(mu[:], stf[:, 0], 1.0 / S)
            nc.vector.tensor_mul(var[:], mu[:], mu[:])
            nc.vector.tensor_scalar_mul(var[:], var[:], -float(S) / (S - 1))
            nc.vector.tensor_scalar_mul(stf[:, 1], stf[:, 1], 1.0 / (S - 1))
            nc.vector.tensor_add(var[:], var[:], stf[:, 1])
            nc.scalar.activation(var[:], var[:], ACT.Sqrt, bias=eps_sb[:])
            nc.vector.reciprocal(r1[:], var[:])
            nc.vector.tensor_mul(n1[:], mu[:], r1[:])
            nc.vector.tensor_scalar_mul(n1[:], n1[:], -1.0)
            for et in range(ET):
                stage = lnp.tile([P, NL], BF, tag="stage")
                for b in range(B):
                    nc.scalar.activation(
                        stage[:, ds(b * SL, SL)], y1sb[:, et, ds(b * SL, SL)],
                        ACT.Identity, bias=n1[:, et, b : b + 1],
                        scale=r1[:, et, b : b + 1],
                    )
                nc.sync.dma_start(out=x1T[et * P : (et + 1) * P, :], in_=stage[:])
        y1_ctx.close()
        qo_ctx.close()
        xsT_ctx.close()

        # ---------- Phase F: FFN1 -> hT ----------
        with ExitStack() as ctxF:
            w1_pool = ctxF.enter_context(tc.tile_pool(name="w_f1", bufs=5))
            x1_pool = ctxF.enter_context(
                tc.tile_pool(name="kxn_x1", bufs=k_pool_min_bufs(x1T[:]))
            )
            kxm_prod, kxm_shape = dma_from_dram_kxm(w1_pool, W1T[:])
            kxn_prod, kxn_shape = dma_from_dram_kxn(x1_pool, x1T[:])
            composable_matmul_tile_kernel(
                tc=tc,
                psum_n_bufs=2,
                kxm_shape=kxm_shape,
                kxn_shape=kxn_shape,
                output_type=BF,
                kxm_producer=kxm_prod,
                kxn_producer=kxn_prod,
                mxn_consumer=dma_to_dram_mxn(hT[:]),
                mxn_subtile_reducer=_bias_act_reducer(b1_sb, ACT.Relu),
            )

        # ---------- Phase G: FFN2 + residual + inline LN2 stats -> y2T ----------
        st2_ctx = ExitStack()
        st2_pool = st2_ctx.enter_context(tc.tile_pool(name="st2sb", bufs=1))
        st2_sb = st2_pool.tile([P, 2, ET, B], F32)
        with ExitStack() as ctxG:
            w2_pool = ctxG.enter_context(
                tc.tile_pool(name="w_f2", bufs=k_pool_min_bufs(W2T[:]))
            )
            hT_pool = ctxG.enter_context(
                tc.tile_pool(name="kxn_hT", bufs=k_pool_min_bufs(hT[:]))
            )
            cons2_pool = ctxG.enter_context(tc.tile_pool(name="f2_cons", bufs=3))
            kxm_prod, kxm_shape = dma_from_dram_kxm(w2_pool, W2T[:])
            kxn_prod, kxn_shape = dma_from_dram_kxn(hT_pool, hT[:])

            def f2_consumer(nc_, mxn_tile, md):
                c = md.n_tile_idx
                w = md.n_slice_size
                for sub in range(md.m_subtiles):
                    m_abs = md.m_tile_idx * md.m_subtiles + sub
                    sl = mxn_tile[:, sub, :w]
                    x1t = cons2_pool.tile([P, 512], BF, tag="f2_res")
                    nc_.sync.dma_start(
                        out=x1t[:, :w],
                        in_=x1T[m_abs * P : (m_abs + 1) * P, ds(c * 512, w)],
                    )
                    nc_.vector.tensor_add(out=sl, in0=sl, in1=x1t[:, :w])
                    nc_.vector.tensor_reduce(
                        out=st2_sb[:, 0, m_abs, 2 * c : 2 * c + 2],
                        in_=sl.rearrange("p (b s) -> p b s", b=2),
                        axis=AX.X,
                        op=ALU.add,
                    )
                    sq = cons2_pool.tile([P, 512], F32, tag="f2_sq")
                    nc_.scalar.activation(sq[:, :w], sl, ACT.Square)
                    nc_.vector.tensor_reduce(
                        out=st2_sb[:, 1, m_abs, 2 * c : 2 * c + 2],
                        in_=sq[:, :w].rearrange("p (b s) -> p b s", b=2),
                        axis=AX.X,
                        op=ALU.add,
                    )
                    nc_.sync.dma_start(
                        out=y2T[m_abs * P : (m_abs + 1) * P, ds(c * 512, w)], in_=sl
                    )

            composable_matmul_tile_kernel(
                tc=tc,
                psum_n_bufs=2,
                kxm_shape=kxm_shape,
                kxn_shape=kxn_shape,
                output_type=F32,
                kxm_producer=kxm_prod,
                kxn_producer=kxn_prod,
                mxn_consumer=f2_consumer,
                mxn_subtile_reducer=_bias_act_reducer(b2_sb, ACT.Identity),
            )

        # ---------- Phase H: LN2 (stats AR + normalize) -> outT ----------
        with ExitStack() as ctxH:
            lnp = ctxH.enter_context(tc.tile_pool(name="ln2", bufs=4))
            nc.sync.dma_start(out=st2_loc[:], in_=st2_sb[:])
            nc.gpsimd.collective_compute(
                "AllReduce", ALU.add, replica_groups=rg,
                ins=[st2_loc[:]], outs=[st2_full[:]],
            )
            stf = lnp.tile([P, 2, ET, B], F32, tag="stf2")
            nc.sync.dma_start(out=stf[:], in_=st2_full[:])
            r2 = lnp.tile([P, ET, B], F32, tag="r2")
            n2 = lnp.tile([P, ET, B], F32, tag="n2")
            mu = lnp.tile([P, ET, B], F32, tag="mu2")
            var = lnp.tile([P, ET, B], F32, tag="var2")
            nc.vector.tensor_scalar_mul(mu[:], stf[:, 0], 1.0 / S)
            nc.vector.tensor_mul(var[:], mu[:], mu[:])
            nc.vector.tensor_scalar_mul(var[:], var[:], -float(S) / (S - 1))
            nc.vector.tensor_scalar_mul(stf[:, 1], stf[:, 1], 1.0 / (S - 1))
            nc.vector.tensor_add(var[:], var[:], stf[:, 1])
            nc.scalar.activation(var[:], var[:], ACT.Sqrt, bias=eps_sb[:])
            nc.vector.reciprocal(r2[:], var[:])
            nc.vector.tensor_mul(n2[:], mu[:], r2[:])
            nc.vector.tensor_scalar_mul(n2[:], n2[:], -1.0)
            for et in range(ET):
                yt = lnp.tile([P, NL], F32, tag="yt2")
                nc.sync.dma_start(out=yt[:], in_=y2T[et * P : (et + 1) * P, :])
                stage = lnp.tile([P, NL], F32, tag="ostage")
                for b in range(B):
                    nc.scalar.activation(
                        stage[:, ds(b * SL, SL)], yt[:, ds(b * SL, SL)],
                        ACT.Identity, bias=n2[:, et, b : b + 1],
                        scale=r2[:, et, b : b + 1],
                    )
                nc.sync.dma_start(out=outT[et * P : (et + 1) * P, :], in_=stage[:])
        st2_ctx.close()

    nc.compile()
    return nc


def _prep_inputs(x, Wq, bq, Wk, bk, Wv, bv, Wo, bo, W1, b1, W2, b2):
    bf = ml_dtypes.bfloat16
    f32 = np.float32

    def cvt(a, dt):
        return np.ascontiguousarray(np.asarray(a), dtype=dt)

    shared = {
        "WqT": cvt(np.asarray(Wq).reshape(HD, E).T, bf),
        "WkT": cvt(np.asarray(Wk).reshape(HD, E).T, bf),
        "WvT": cvt(np.asarray(Wv).reshape(HD, E).T, bf),
        "WoT": cvt(np.asarray(Wo).T, bf),
        "W1T": cvt(np.asarray(W1).T, bf),
        "W2T": cvt(np.asarray(W2).T, bf),
        "bq_c": cvt(np.asarray(bq).T, f32),
        "bk_c": cvt(np.asarray(bk).T, f32),
        "bv_r": cvt(np.asarray(bv).reshape(1, HD), f32),
        "bo_c": cvt(np.asarray(bo).reshape(ET, P).T, f32),
        "b1_c": cvt(np.asarray(b1).reshape(FT, P).T, f32),
        "b2_c": cvt(np.asarray(b2).reshape(ET, P).T, f32),
    }
    x = np.asarray(x, dtype=f32)
    in_maps = []
    for r in range(R):
        m = dict(shared)
        m["x_s"] = np.ascontiguousarray(x[:, r * SL : (r + 1) * SL, :])
        in_maps.append(m)
    return in_maps


def kernel(x, Wq, bq, Wk, bk, Wv, bv, Wo, bo, W1, b1, W2, b2):
    global LAST_EXEC_NS
    if "nc" not in _STATE:
        _STATE["nc"] = build()
    nc = _STATE["nc"]

    in_maps = _prep_inputs(x, Wq, bq, Wk, bk, Wv, bv, Wo, bo, W1, b1, W2, b2)
    trace = os.environ.get("KERNEL_TRACE", "0") == "1"
    if trace:
        _install_ntff_hook()
    try:
        res = run_bass_kernel_spmd(nc, in_maps, core_ids=list(range(R)), trace=trace)
    except Exception:
        if not trace:
            raise
        res = run_bass_kernel_spmd(nc, in_maps, core_ids=list(range(R)), trace=False)
    LAST_EXEC_NS = res.exec_time_ns
    _STATE["last_res"] = res

    parts = [
        res.results[r]["outT"].reshape(E, B, SL).transpose(1, 2, 0) for r in range(R)
    ]
    return np.ascontiguousarray(np.concatenate(parts, axis=1), dtype=np.float32)

